# revision 28
# baseline (speedup 1.0000x reference)
"""ConvInsert Trainium2 kernel (8-core data-parallel).

Problem: input (32, 256, 4096) f32. Each row of 4096 is 512 slices of 8.
For each of the 511 adjacent slice pairs (a 16-element window), two dot
products (with w1, w2) plus bias are inserted after the first slice:
output rows are 511*10 + 8 = 5118 wide.

Strategy (memory-regime):
  - Shard batch dim over 8 cores: per core x[1024, 4096] -> y[1024, 5118].
  - fp16 end-to-end: the host casts x to fp16 before upload and widens y
    back to f32 after. Halves HBM traffic (the roofline for this memory-
    regime problem) and makes PE matmuls single-pass (f32 matmuls run as
    hi/lo pairs) and DVE copies 2x. Data range is ~N(0,1) so fp16 keeps
    the scale-relative error ~1e-3, well inside the gate.
  - Rows on SBUF partitions; fully contiguous DMA in (8KB/row) and
    out (10.2KB/row).
  - The 1022 inserted values per row are computed on TensorE: per
    128-column chunk q, transpose the chunk (PE transpose), then one
    matmul xT_q.T-contraction against a host-prepared [128, 34]
    coefficient matrix L accumulates window dot products directly into
    PSUM laid out as [128 rows, 1022 = (window, which-w)].
  - DVE does one big strided copy to interleave the 512 slices into the
    output tile, and one tensor_add to scatter PSUM results + bias into
    the insert positions.
"""

import numpy as np

import concourse.bass as bass
import concourse.mybir as mybir
import concourse.tile as tile_mod
from bass_rust import ScopedClock
from concourse.tile import TileContext
from concourse.bass_utils import run_bass_kernel_spmd

F32 = mybir.dt.float32
F16 = mybir.dt.float16

N_CORES = 8
BATCHES = 32
CH = 256
IN_COLS = 4096
KSIZE = 16
HALF = 8
N_SLICES = IN_COLS // HALF            # 512
N_WIN = N_SLICES - 1                  # 511
OUT_COLS = N_WIN * (HALF + 2) + HALF  # 5118
ROWS_PER_CORE = (BATCHES // N_CORES) * CH  # 1024
N_TILES = ROWS_PER_CORE // 128        # 8
N_CHUNKS = IN_COLS // 128             # 32


# ---------------------------------------------------------------------------
# Workaround: this walrus build rejects CTRL instructions with >1 sync wait.
# TileContext's final drain waits on every outstanding proc sem at once;
# split those waits across single-wait NOPs on SP (executed in order, so the
# barrier/sem-clear that follows still happens after everything completes).
# ---------------------------------------------------------------------------
def _patched_drain_and_barrier(self, tick_clock, wait_clock):
    nc = self.nc
    drain_inst = nc.sync.drain()
    wait_clock.add_sem_waits(
        drain_inst.ins, ScopedClock({None: tick_clock.global_clock})
    )
    si = drain_inst.ins.sync_info
    waits = list(si.on_wait or []) if si is not None else []
    if len(waits) > 1:
        si.on_wait = []
        assert self.sems is not None
        by_name = {h.name: h for h in self.sems.allocated().values()}
        for sw in waits:
            h = by_name[sw.ant_name]
            op = sw.wait_mode[:-4] if sw.wait_mode.endswith("-imm") else sw.wait_mode
            nc.sync.nop().wait_op(h, sw.wait_value, op)

    nc.all_engine_barrier()
    assert self.sems is not None
    popped = nc._tile_sem_poison_stack.pop()
    assert popped is self._sem_poison
    nc.clear_and_free_semaphores(list(self.sems.allocated().values()))
    nc.all_engine_barrier()


tile_mod.TileContext._drain_and_barrier = _patched_drain_and_barrier


def _split_multi_waits(nc):
    """Walrus here allows one sync-wait per instruction: hoist extra
    semaphore waits onto same-engine NOPs placed immediately before the
    instruction (sequencers execute in order, so semantics are identical)."""
    for f in nc.m.functions:
        for bb in f.blocks:
            new_insts = []
            changed = False
            for inst in bb.instructions:
                si = inst.sync_info
                waits = list(si.on_wait) if (si is not None and si.on_wait) else []
                if len(waits) > 1:
                    sem_waits = [w for w in waits if w.sync_type == "semaphore"]
                    other = [w for w in waits if w.sync_type != "semaphore"]
                    keep_n = 0 if other else 1
                    moved = sem_waits[: len(sem_waits) - keep_n]
                    kept = other + sem_waits[len(sem_waits) - keep_n :]
                    if moved:
                        changed = True
                        for sw in moved:
                            nop = mybir.InstNoOp(
                                name=f"wsplit-{nc.next_id()}", ins=[], outs=[]
                            )
                            nop.engine = inst.engine
                            nop.sync_info = mybir.SyncInfo(
                                on_wait=[sw], on_update=[]
                            )
                            new_insts.append(nop)
                        si.on_wait = kept
                new_insts.append(inst)
            if changed:
                bb.instructions = new_insts


def _build_nc():
    nc = bass.Bass()
    x = nc.declare_dram_parameter("x", [ROWS_PER_CORE, IN_COLS], F16, isOutput=False)
    # consts packed into 2 blobs -> 2 DMA issues: cmat = [lmat | ident],
    # vrow = [ones | bias_row]
    cmat = nc.declare_dram_parameter("cmat", [128, 34 + 128], F16, isOutput=False)
    vrow = nc.declare_dram_parameter(
        "vrow", [1, 128 + 2 * N_WIN], F16, isOutput=False
    )
    y = nc.declare_dram_parameter("y", [ROWS_PER_CORE, OUT_COLS], F16, isOutput=True)

    with TileContext(nc) as tc:
        with (
            tc.tile_pool(name="const", bufs=1) as cpool,
            tc.tile_pool(name="xin", bufs=8) as xpool,
            tc.tile_pool(name="xt", bufs=3) as xtpool,
            tc.tile_pool(name="outb", bufs=3) as opool,
            tc.tile_pool(name="pst", bufs=2, space="PSUM") as pst_pool,
            tc.tile_pool(name="pso", bufs=3, space="PSUM") as pso_pool,
        ):
            # Consts: 2 issues on scalar's HWDGE queue, concurrent with sync's
            # x-tile issues below.
            cmat_sb = cpool.tile([128, 34 + 128], F16)
            nc.scalar.dma_start(out=cmat_sb[:], in_=cmat[:, :])
            vrow_sb = cpool.tile([1, 128 + 2 * N_WIN], F16)
            nc.scalar.dma_start(out=vrow_sb[:], in_=vrow[:, :])


            # all x tile loads up front on sync's queue
            x_sbs = []
            for t in range(N_TILES):
                x_sb = xpool.tile([128, IN_COLS], F16, tag="x")
                nc.sync.dma_start(out=x_sb[:], in_=x[t * 128 : (t + 1) * 128, :])
                x_sbs.append(x_sb)

            for t in range(N_TILES):
                rows = slice(t * 128, (t + 1) * 128)
                x_sb = x_sbs[t]

                out_sb = opool.tile([128, OUT_COLS + 2], F16, tag="o")
                out_ps = pso_pool.tile([128, 1024], F32, tag="ops")

                # seed PSUM with the bias via two K=1 rank-1 matmuls
                # (ones[1,128].T @ bias_row chunk); the window matmuls then
                # accumulate on top. Kills the 523KB broadcast-bias constant.
                nc.tensor.matmul(
                    out_ps[:, 0:512], vrow_sb[:, 0:128],
                    vrow_sb[:, 128 : 128 + 512],
                    start=True, stop=False, skip_group_check=True,
                )
                nc.tensor.matmul(
                    out_ps[:, 512 : 2 * N_WIN], vrow_sb[:, 0:128],
                    vrow_sb[:, 128 + 512 : 128 + 2 * N_WIN],
                    start=True, stop=False, skip_group_check=True,
                )

                # per-chunk compute: (psum col range, L col range, first-in-bank)
                def _mm_plan(q):
                    if q == 0:
                        return [(0, 32, 2, 34, True, False)]
                    if q == 16:
                        return [
                            (510, 512, 0, 2, False, True),
                            (512, 544, 2, 34, True, False),
                        ]
                    if q == N_CHUNKS - 1:
                        return [(32 * q - 2, 32 * q + 30, 0, 32, False, True)]
                    return [(32 * q - 2, 32 * q + 32, 0, 34, False, False)]

                # 8-chunk super-groups: fewer, bigger ACT copies (PSUM drain)
                for g in range(N_CHUNKS // 8):
                    xt_ps = pst_pool.tile([128, 1024], F16, tag="xtp")
                    for k in range(8):
                        q = 8 * g + k
                        nc.tensor.transpose(
                            xt_ps[:, 128 * k : 128 * (k + 1)],
                            x_sb[:, 128 * q : 128 * (q + 1)],
                            cmat_sb[:, 34 : 34 + 128],
                        )
                    xt_sb = xtpool.tile([128, 1024], F16, tag="xts")
                    nc.scalar.copy(out=xt_sb[:], in_=xt_ps[:])
                    for k in range(8):
                        q = 8 * g + k
                        for (c0, c1, l0, l1, first, stop) in _mm_plan(q):
                            for h in (0, 1):
                                nc.tensor.matmul(
                                    out_ps[64 * h : 64 * (h + 1), c0:c1],
                                    xt_sb[:, 128 * k + 64 * h
                                          : 128 * k + 64 * (h + 1)],
                                    cmat_sb[:, l0:l1],
                                    start=False,
                                    stop=(stop and h == 1),
                                    skip_group_check=True,
                                    tile_position=(0, 64 * h),
                                )

                # interleave the 512 slices into the output layout
                src = x_sb.rearrange("p (i j) -> p i j", j=HALF)
                out_v = out_sb.rearrange("p (i j) -> p i j", j=HALF + 2)
                nc.vector.tensor_copy(out=out_v[:, :, 0:HALF], in_=src)

                # scatter inserted values (bias already accumulated in PSUM)
                nc.vector.tensor_copy(
                    out=out_v[:, 0:N_WIN, HALF : HALF + 2],
                    in_=out_ps[:, 0 : 2 * N_WIN].rearrange("p (i t) -> p i t", t=2),
                )

                # Output via SWDGE on GpSimd: its own DMA queue, and the issue
                # doesn't sit on ACT's in-order queue blocking the next tile's
                # PSUM drains while it waits for the scatter.
                nc.gpsimd.dma_start(out=y[rows, :], in_=out_sb[:, 0:OUT_COLS])

    _split_multi_waits(nc)
    return nc


_NC_CACHE = {}


def _get_nc():
    if "nc" not in _NC_CACHE:
        _NC_CACHE["nc"] = _build_nc()
    return _NC_CACHE["nc"]


def _build_lmat(w1, w2):
    """L[c, d]: coefficient of x[p, 128q + c] in psum column block d.

    Column d=t in {0,1}: B-part (second half-window) contribution of this
    chunk's first slice to the previous chunk's last window.
    Columns d = 2 + 2*wl + t: window (16q + wl), weight-vector t.
    """
    L = np.zeros((128, 34), dtype=np.float32)
    for tt, wv in enumerate((w1, w2)):
        wv = np.asarray(wv, dtype=np.float32).reshape(KSIZE)
        L[0:HALF, tt] = wv[HALF:]
        for wl in range(16):
            d = 2 + 2 * wl + tt
            lo = HALF * wl
            hi = min(lo + KSIZE, 128)
            L[lo:hi, d] = wv[: hi - lo]
    return L


_LAST_BKR = [None]


def kernel(inputs, w1, w2, b, _trace=False, _trace_kwargs=None):
    inputs = np.asarray(inputs, dtype=np.float32).astype(np.float16)
    L = _build_lmat(w1, w2).astype(np.float16)
    cmat = np.concatenate([L, np.eye(128, dtype=np.float16)], axis=1)
    cmat = np.ascontiguousarray(cmat, dtype=np.float16)
    bias_row = np.asarray(b, dtype=np.float32).reshape(1, 2 * N_WIN)
    vrow = np.concatenate(
        [np.ones((1, 128), dtype=np.float16), bias_row.astype(np.float16)],
        axis=1,
    )
    vrow = np.ascontiguousarray(vrow, dtype=np.float16)

    per_core = BATCHES // N_CORES
    in_maps = []
    for c in range(N_CORES):
        xc = inputs[c * per_core : (c + 1) * per_core].reshape(
            ROWS_PER_CORE, IN_COLS
        )
        in_maps.append({"x": np.ascontiguousarray(xc), "cmat": cmat, "vrow": vrow})

    nc = _get_nc()
    kwargs = {}
    if _trace:
        kwargs["trace"] = True
        if _trace_kwargs:
            kwargs.update(_trace_kwargs)
    bkr = run_bass_kernel_spmd(nc, in_maps, list(range(N_CORES)), **kwargs)
    _LAST_BKR[0] = bkr
    out = np.empty((BATCHES, CH, OUT_COLS), dtype=np.float32)
    for c in range(N_CORES):
        out[c * per_core : (c + 1) * per_core] = (
            bkr.results[c]["y"].astype(np.float32).reshape(per_core, CH, OUT_COLS)
        )
    return out

